# revision 65
# baseline (speedup 1.0000x reference)
"""Single-head cross-attention on 8 NeuronCores, data-parallel over batch.

Math per core (batch element b):
    q = x @ Wq + bq;  k = enc @ Wk + bk;  v = enc @ Wv + bv
    out = softmax(q k^T / sqrt(H)) @ v @ Wp + bp

Layout strategy (no on-chip transposes anywhere):
    host:    xT[E,T], encT[E,S] (pre-transposed), Wq' = Wq/sqrt(H)
    qT[h,t] = Wq'^T-tiles as lhsT, xT as rhs          (+bq' per-partition)
    kT[h,s] = Wk-tiles  as lhsT, encT as rhs          (+bk  per-partition)
    v[s,h]  = encT-tiles as lhsT, Wv as rhs
    ST[s,t] = kT-tiles  as lhsT, qT as rhs            (scores, transposed)
    Ex      = exp(ST)          (no max subtraction; scores are O(1) here,
                                softmax is shift-invariant so result matches)
    r[t]    = ones^T column matmuls over Ex s-tiles   ([t,1] per t-tile)
    OT[h,t] = v-tiles   as lhsT, Ex as rhs            (unnormalized)
    Y[t,e]  = OT-tiles  as lhsT, Wp as rhs, scaled by 1/r[t] on copy-out
    bv/bp are folded into a host-side rank-1 add: softmax rows sum to 1, so
    P@(v + 1 bv^T) @ Wp + bp = P@v@Wp + 1 (bv@Wp + bp)^T exactly.

All matmul operands are bf16 (1 col/cycle on the PE like fp32r, but half
the DMA bytes and 4x-faster FWL weight loads; measured 13us faster than
the fp32r build end-to-end). PSUM accumulation stays fp32.

DMA: operands are HOST-PACKED into per-j interleaved tensors
(wq_j|xT_j), (wk_j|encT_j) so one ~450KB trigger delivers exactly one
j-round of accumulation operands in consumption order, with the
(padded) bias columns spliced in after j=0 so they ride trigger 2
instead of paying their own descriptor-heavy gather (a [128,12] gather
is ~1040 tiny packets and measurably stalls the queue). Triggers issue
serially at ~650ns on Sync and each pays ~1us queue doorbell +
~320GB/s aggregate + >1us completion-sem latency, so few+big+in-order
wins; 15 input triggers total (was 33). Input triggers stay on Sync:
Scalar's first use pays a 1.3us ACT_TABLE_LOAD and GpSimd-issued DMAs
showed multi-us queue drains.

ONE psum pool spans the whole kernel (tag "mm" = three [P,T] slots, tag
"r" = one slot used as each qk-pass's third accumulator and later the
row-sum accumulator). A psum pool-scope close is a barrier on ALL prior
drains — two scoped pools cost a measured 1.6us at the phase boundary;
a single rotating pool pays only per-slot WAR deps.

Startup: the PE HAM clock-gate needs ~3.4us of gapless matmul activity
to lift the 1.2->2.4GHz throttle, and the first packed trigger's
completion sem lands ~11.2us, so eight 512-col junk warmup matmuls
(operands memset on-chip, output region overwritten by the first real
accumulation's start=True) bridge engine-GO (~7.6us) to data-ready with
no idle gap; the real matmuls then run warm from the start.

The q/k projection phases accumulate j-outer across six packed 512-col
PSUM regions so the tensor engine consumes each (wq_j, xT_j) e-tile as
it lands; drains alternate vector/scalar so pass boundaries don't
serialize on one engine. Output returns as bf16 (cast to fp32 on
host); the last proj tile's 1/r scaling is split across both engines
before the out-DMA so only a 256-col scalar drain + one out-DMA trail
the final matmul.
"""

import os

import numpy as np
import ml_dtypes

import concourse.bass as bass
import concourse.bacc as bacc
import concourse.tile as tile
from concourse import mybir
from concourse.bass_utils import run_bass_kernel_spmd

P = 128
B, T, S, E, H = 8, 1024, 1024, 768, 768
NE, NH, NT, NS = E // P, H // P, T // P, S // P
F32 = mybir.dt.float32
BF16 = mybir.dt.bfloat16
MM_DT = mybir.dt.bfloat16
AFT = mybir.ActivationFunctionType
JW = H + T  # packed per-j stride: weight tile (768) + activation tile (1024)
BQW = 16    # bias columns (2*NH=12, padded) spliced into packA after j=0

_NC_CACHE = {}
LAST_RESULT = None


def _build_bass():
    nc = bacc.Bacc()
    packA_d = nc.declare_dram_parameter(
        "packA", [P, NE * JW + BQW], MM_DT, isOutput=False)
    packB_d = nc.declare_dram_parameter("packB", [P, NE * JW], MM_DT, isOutput=False)
    wv_d = nc.declare_dram_parameter("wv", [P, NE * H], MM_DT, isOutput=False)
    wp_d = nc.declare_dram_parameter("wp", [P, NH * E], MM_DT, isOutput=False)
    out_d = nc.declare_dram_parameter("out", [T, E], BF16, isOutput=True)
    rrow_d = nc.dram_tensor("rrow_bounce", [1, T], F32)

    def mm(ps, lhsT, rhs, start, stop):
        nc.tensor.matmul(ps, lhsT, rhs, start=start, stop=stop)

    with tile.TileContext(nc) as tc:
        with (
            tc.tile_pool(name="sb", bufs=1) as sbp,
            tc.tile_pool(name="yout", bufs=3) as youtp,
        ):
            ones_sb = sbp.tile([P, 2], MM_DT, tag="ones")
            nc.vector.memset(ones_sb[:], 1.0)
            # picks partitions {0,32,64,96} when used as lhsT: reduces the
            # four col-tiled row-sum partials in one matmul contraction
            rmask_sb = sbp.tile([P, 2], MM_DT, tag="rmask")
            nc.vector.memset(rmask_sb[:], 0.0)
            for g in range(4):
                nc.vector.memset(rmask_sb[32 * g:32 * g + 1, :], 1.0)
            rcp_sb = sbp.tile([P, NT], F32, tag="rcp")
            rrow_sb = sbp.tile([P, T], F32, tag="rrow")

            # packed phase-1 operands + long-lived activations
            pa_sb = sbp.tile([P, NE * JW + BQW], MM_DT, tag="packA")
            bqk_sb = sbp.tile([P, 2 * NH], F32, tag="bqk")
            pb_sb = sbp.tile([P, NE * JW], MM_DT, tag="packB")
            wv_sb = sbp.tile([P, NE * H], MM_DT, tag="wv")
            wp_sb = sbp.tile([P, NH * E], MM_DT, tag="wp")
            qT_sb = sbp.tile([P, NH * T], MM_DT, tag="qT")
            kT_sb = sbp.tile([P, NH * S], MM_DT, tag="kT")
            v_sb = sbp.tile([P, NS * H], MM_DT, tag="v")
            ex_sb = sbp.tile([P, NS * T], MM_DT, tag="ex")
            ot_sb = sbp.tile([P, NH * T], MM_DT, tag="ot")

            # All input triggers on Sync (other engines pay their own
            # first-use penalties: Scalar a 1.3us ACT_TABLE_LOAD, GpSimd a
            # multi-us queue DRAIN — measured net losses). One trigger per
            # j-round of (weight|activation) in consumption order; each
            # transfer pays ~1us queue startup + ~320GB/s aggregate + ~0.7us
            # completion-sem latency, so the first trigger carries exactly
            # the first accumulation round's operands.
            warm_src = sbp.tile([P, 512], MM_DT, tag="warm")
            nc.gpsimd.memset(warm_src[:], 0.0)
            def patrig(j):
                nc.sync.dma_start(
                    pa_sb[:, BQW + j * JW:BQW + (j + 1) * JW],
                    packA_d[:, BQW + j * JW:BQW + (j + 1) * JW])

            nc.sync.dma_start(pa_sb[:, 0:H + 512], packA_d[:, 0:H + 512])
            patrig(1)
            patrig(2)
            # xT0's second half + bias cols: first needed by the h0=512 pass
            nc.sync.dma_start(pa_sb[:, H + 512:JW + BQW],
                              packA_d[:, H + 512:JW + BQW])
            for j in range(3, NE):
                patrig(j)
            # bias cols ride trigger 2 as bf16; widen once for the drains
            nc.vector.tensor_copy(bqk_sb[:], pa_sb[:, JW:JW + 2 * NH])
            for j in range(NE):
                nc.sync.dma_start(pb_sb[:, j * JW:(j + 1) * JW],
                                  packB_d[:, j * JW:(j + 1) * JW])
            nc.sync.dma_start(wv_sb[:], wv_d[:])
            nc.sync.dma_start(wp_sb[:], wp_d[:])

            def wtile(pk, j, i, bump=True):
                o = BQW if (bump and j >= 1) else 0
                return pk[:, o + j * JW + i * P: o + j * JW + (i + 1) * P]

            def atile(pk, j, c0, c1, bump=True):
                o = BQW if (bump and j >= 1) else 0
                return pk[:, o + j * JW + H + c0: o + j * JW + H + c1]

            # ONE psum pool for the whole kernel: a 3-slot ring of [P,T]
            # tiles (tag mm, 6 banks) + a 1-slot tag r (2 banks) that phase 1
            # borrows as each qk-pass's third accumulator and phase 2 uses
            # for the row-sum accumulation. A pool-scope close is a barrier
            # (the next pool's first tile waits on ALL prior drains — cost
            # a measured 1.6us at the phase boundary); one pool rotates
            # straight through with only per-slot WAR deps.
            with tc.tile_pool(name="ps", bufs=3, space="PSUM") as psq:
                # qT / kT: j-outer accumulation so the PE consumes operand
                # e-tiles in DMA-arrival order. Six 512-col accumulation
                # regions live at once, packed two per [P, T] psum tile;
                # the drains alternate vector/scalar so neither engine gates.
                first_pass = [True]

                def proj_qk(pk_sb, dst_sb, b_cols, width, bump):
                    for h0 in range(0, width, 512):
                        pst = [psq.tile([P, T], F32, tag="mm", name=f"qk{h0}_{u}")
                               for u in range(2)]
                        pst.append(psq.tile([P, T], F32, tag="r", bufs=1,
                                            name=f"qkr{h0}"))
                        if first_pass[0]:
                            # PE p-state warm-up while the first DMAs land:
                            # junk matmuls with no DMA deps, overwritten by
                            # the q accumulation's start=True below.
                            first_pass[0] = False
                            # rotate junk writes across all SIX accumulation
                            # regions: with only 2 regions, warmup n waits
                            # warmup n-2's ~650ns PSUM write-ack with <1us
                            # slack — occasional hiccups reset the HAM busy
                            # window and leave early real matmuls at 1.2GHz
                            for w in range(9):
                                nc.tensor.matmul(
                                    pst[w % 3][0:2, (w % 2) * 512:
                                               (w % 2) * 512 + 512],
                                    ones_sb[:], warm_src[:],
                                    start=True, stop=True)
                        def acc(i):
                            return pst[i // 2][:, (i % 2) * 512:(i % 2) * 512 + 512]
                        for j in range(NE):
                            iorder = range(NH) if j < NE - 1 else range(NH - 1, -1, -1)
                            for i in iorder:
                                mm(acc(i),
                                   wtile(pk_sb, j, i, bump),
                                   atile(pk_sb, j, h0, h0 + 512, bump),
                                   start=(j == 0), stop=(j == NE - 1))
                        for i in range(NH - 1, -1, -1):
                            dst = dst_sb[:, i * width + h0: i * width + h0 + 512]
                            if i % 2 == 0:
                                nc.vector.tensor_scalar_add(
                                    dst, acc(i), b_cols[i])
                            else:
                                nc.scalar.activation(
                                    dst, acc(i), AFT.Identity,
                                    bias=b_cols[i])

                proj_qk(pa_sb, qT_sb,
                        [bqk_sb[:, i:i + 1] for i in range(NH)], T, True)
                proj_qk(pb_sb, kT_sb,
                        [bqk_sb[:, NH + i:NH + i + 1] for i in range(NH)], S,
                        False)

                # v[s-tile si] = sum_j encT[e_j, s_si]^T @ Wv[e_j, :]
                for si in range(NS):
                    ps = psq.tile([P, T], F32, tag="mm")
                    for n0, n1 in ((0, 512), (512, H)):
                        for j in range(NE):
                            mm(ps[:, n0:n1],
                               atile(pb_sb, j, si * P, (si + 1) * P, False),
                               wv_sb[:, j * H + n0: j * H + n1],
                               start=(j == 0), stop=(j == NE - 1))
                    if si == NS - 1:
                        nc.scalar.copy(v_sb[:, si * H:si * H + 384], ps[:, 0:384])
                        nc.vector.tensor_copy(
                            v_sb[:, si * H + 384:(si + 1) * H], ps[:, 384:H])
                    elif si % 2 == 0:
                        nc.scalar.copy(v_sb[:, si * H:(si + 1) * H], ps[:, 0:H])
                    else:
                        nc.vector.tensor_copy(
                            v_sb[:, si * H:(si + 1) * H], ps[:, 0:H])

                # ST[s-tile si] = sum_i kT[h_i, s_si]^T @ qT[h_i, :]; Ex = exp
                for si in range(NS):
                    ps = psq.tile([P, T], F32, tag="mm")
                    for h0 in range(0, T, 512):
                        for i in range(NH):
                            mm(ps[:, h0:h0 + 512],
                               kT_sb[:, i * S + si * P: i * S + (si + 1) * P],
                               qT_sb[:, i * T + h0: i * T + h0 + 512],
                               start=(i == 0), stop=(i == NH - 1))
                    nc.scalar.activation(
                        ex_sb[:, si * T:(si + 1) * T], ps[:], AFT.Exp)

                # Pre-sum Ex si-pairs on the (idle) Vector engine so the
                # row-sum pass needs only ONE matmul per (col-group, h0) —
                # two independent start+stop waves of 4 concurrent matmuls,
                # no accumulation continuation between waves (the k0->k1
                # continuation cost a measured 835ns ack bubble).
                exsum_sb = sbp.tile([P, 4 * T], MM_DT, tag="exsum")
                for g in range(4):
                    nc.vector.scalar_tensor_tensor(
                        exsum_sb[:, g * T:(g + 1) * T],
                        ex_sb[:, 2 * g * T:(2 * g + 1) * T], 1.0,
                        ex_sb[:, (2 * g + 1) * T:(2 * g + 2) * T],
                        mybir.AluOpType.mult, mybir.AluOpType.add)

                # OT[h-tile i] = sum_si v[s_si, h_i]^T @ Ex[s_si, :]
                for i in range(NH):
                    ps = psq.tile([P, T], F32, tag="mm")
                    for h0 in range(0, T, 512):
                        for si in range(NS):
                            mm(ps[:, h0:h0 + 512],
                               v_sb[:, si * H + i * P: si * H + (i + 1) * P],
                               ex_sb[:, si * T + h0: si * T + h0 + 512],
                               start=(si == 0), stop=(si == NS - 1))
                    if i == NH - 1:
                        # split the LAST drain across both engines: the Y
                        # phase's first tile stalls on it otherwise
                        nc.scalar.copy(ot_sb[:, i * T:i * T + 512],
                                       ps[:, 0:512])
                        nc.vector.tensor_copy(
                            ot_sb[:, i * T + 512:(i + 1) * T], ps[:, 512:T])
                    elif i % 2 == 0:
                        nc.scalar.copy(ot_sb[:, i * T:(i + 1) * T], ps[:])
                    else:
                        nc.vector.tensor_copy(ot_sb[:, i * T:(i + 1) * T], ps[:])
                    if i == 1:
                        # row-sum waves: 4 concurrent col-tiled matmuls per
                        # h0 half, partial row pairs at psum partitions 32g.
                        # Emitted here so the exsum adds (which trail the
                        # last exp) are long done when the PE arrives.
                        pr4 = psq.tile([P, T], F32, tag="r", bufs=1)
                        for h0 in range(0, T, 512):
                            for g in range(4):
                                nc.tensor.matmul(
                                    pr4[32 * g:32 * g + 2, h0:h0 + 512],
                                    ones_sb[:],
                                    exsum_sb[:, g * T + h0: g * T + h0 + 512],
                                    start=True, stop=True,
                                    tile_position=(0, 32 * g))
                        s4_sb = sbp.tile([P, T], MM_DT, tag="s4")
                        nc.vector.tensor_copy(s4_sb[:], pr4[:])
                    if i == 3:
                        # reduce the 4 col-tiled partials (rows 0/32/64/96
                        # of s4) into the final row-sum with one masked
                        # 128-contraction matmul pair
                        prf = psq.tile([2, T], F32, tag="r", bufs=1)
                        for h0 in range(0, T, 512):
                            nc.tensor.matmul(
                                prf[:, h0:h0 + 512], rmask_sb[:],
                                s4_sb[:, h0:h0 + 512],
                                start=True, stop=True)
                        nc.vector.reciprocal(rrow_sb[0:1, :], prf[0:1, :])

                # scatter the reciprocal row [1, T] into per-partition
                # columns [128, NT] entirely off the PE: bounce the row to
                # DRAM and gather it back partition-strided. Runs ~35us
                # before the proj phase needs rcp, so the latency (~2us,
                # 1024 4-byte read packets) is fully hidden.
                nc.sync.dma_start(rrow_d[:], rrow_sb[0:1, :])
                nc.sync.dma_start(
                    rcp_sb[:, 0:NT],
                    rrow_d[0].rearrange("(ti p) -> p ti", p=P))

                # Y[t-tile ti] = (sum_i OT[h_i, t_ti]^T @ Wp[h_i, :]) * rcp[ti]
                # Alternate the 1/r scaling between vector and scalar; the
                # last tile is split across both engines + two out-DMAs so
                # the tail critical chain after the final matmul is short.
                for ti in range(NT):
                    ps = psq.tile([P, E], F32, tag="mm")
                    last = ti == NT - 1
                    for n0, n1 in ((0, 512), (512, E)):
                        for i in range(NH):
                            mm(ps[:, n0:n1],
                               ot_sb[:, i * T + ti * P: i * T + (ti + 1) * P],
                               wp_sb[:, i * E + n0: i * E + n1],
                               start=(i == 0), stop=(i == NH - 1))
                    if last:
                        # SEPARATE output tiles for the two drain pieces:
                        # writes to one shared y tile serialize WAW (subtile
                        # dep tracking doesn't split them — every prior tail
                        # variant was secretly serial). With separate tiles
                        # Scalar's 256-col drain runs in PARALLEL with
                        # Vector's, gated only by its own PSUM write-ack.
                        y_a = youtp.tile([P, 512], BF16, tag="y")
                        y_b = youtp.tile([P, E - 512], BF16, tag="y")
                        nc.vector.tensor_scalar_mul(
                            y_a[:], ps[:, 0:512], rcp_sb[:, ti:ti + 1])
                        nc.sync.dma_start(
                            out_d[ti * P:(ti + 1) * P, 0:512], y_a[:])
                        nc.scalar.activation(
                            y_b[:], ps[:, 512:E], AFT.Copy,
                            scale=rcp_sb[:, ti:ti + 1])
                        nc.sync.dma_start(
                            out_d[ti * P:(ti + 1) * P, 512:E], y_b[:])
                    else:
                        y_sb = youtp.tile([P, E], BF16, tag="y")
                        if ti % 2 == 0:
                            nc.vector.tensor_scalar_mul(
                                y_sb[:], ps[:], rcp_sb[:, ti:ti + 1])
                        else:
                            nc.scalar.activation(
                                y_sb[:], ps[:], AFT.Copy,
                                scale=rcp_sb[:, ti:ti + 1])
                        nc.sync.dma_start(out_d[ti * P:(ti + 1) * P, :], y_sb[:])
    nc.finalize()
    return nc


def get_nc():
    if "nc" not in _NC_CACHE:
        _NC_CACHE["nc"] = _build_bass()
    return _NC_CACHE["nc"]


def kernel(**inputs):
    global LAST_RESULT
    x = np.asarray(inputs["x"], dtype=np.float32)
    enc = np.asarray(inputs["encoder_out"], dtype=np.float32)
    Wq = np.asarray(inputs["Wq"], dtype=np.float32)
    bq = np.asarray(inputs["bq"], dtype=np.float32)
    Wk = np.asarray(inputs["Wk"], dtype=np.float32)
    bk = np.asarray(inputs["bk"], dtype=np.float32)
    Wv = np.asarray(inputs["Wv"], dtype=np.float32)
    bv = np.asarray(inputs["bv"], dtype=np.float32)
    Wp = np.asarray(inputs["Wp"], dtype=np.float32)
    bp = np.asarray(inputs["bp"], dtype=np.float32)

    bf = ml_dtypes.bfloat16
    scale = np.float32(1.0 / np.sqrt(H))
    wq_s = (Wq * scale).astype(bf)
    bq_s = (bq * scale).astype(np.float32)
    cvec = (bv @ Wp + bp).astype(np.float32)  # exact rank-1 fold, see header
    bqk = np.zeros((P, BQW), dtype=bf)
    bqk[:, 0:NH] = bq_s.reshape(NH, P).T.astype(bf)
    bqk[:, NH:2 * NH] = bk.reshape(NH, P).T.astype(bf)
    xT = np.ascontiguousarray(x.transpose(0, 2, 1)).astype(bf)
    encT = np.ascontiguousarray(enc.transpose(0, 2, 1)).astype(bf)
    wk_b = Wk.astype(bf)
    wv_pk = np.ascontiguousarray(np.concatenate(
        [Wv[j * P:(j + 1) * P, :] for j in range(NE)], axis=1)).astype(bf)
    wp_pk = np.ascontiguousarray(np.concatenate(
        [Wp[j * P:(j + 1) * P, :] for j in range(NH)], axis=1)).astype(bf)

    def packAB(w, act, bias=None):  # per-j: [w_j (768) | act_j (1024)]
        blocks = []
        for j in range(NE):
            blocks.append(np.concatenate(
                [w[j * P:(j + 1) * P, :], act[j * P:(j + 1) * P, :]], axis=1))
            if j == 0 and bias is not None:
                blocks.append(bias)  # bias cols ride trigger 2
        return np.ascontiguousarray(np.concatenate(blocks, axis=1))

    nc = get_nc()
    in_maps = [
        {"packA": packAB(wq_s, xT[i], bqk), "packB": packAB(wk_b, encT[i]),
         "wv": wv_pk, "wp": wp_pk}
        for i in range(B)
    ]
    res = run_bass_kernel_spmd(
        nc, in_maps, list(range(B)),
        trace=bool(os.environ.get("KERNEL_TRACE")),
    )
    LAST_RESULT = res
    out = np.stack([res.results[i]["out"] for i in range(B)]).astype(np.float32)
    if cvec.any():
        out = out + cvec
    return out
